# revision 1
# baseline (speedup 1.0000x reference)
"""Multi-head attention (B=2, S=2048, D=1024, H=16, causal) on 8 TRN2 NeuronCores.

Sharding: core c -> (batch b = c//4, head-group g = c%4, heads 4g..4g+3).
Each core computes Q/K/V projections for its 4 heads, causal flash-style
attention, and a partial output projection (its 256 d_model columns of the
ctx @ wo.T contraction).  Host sums the 4 partials per batch and adds bo.

Device layout: "transposed" activations (feature dim on SBUF partitions,
sequence on the free axis) so every matmul contraction runs along
partitions; host passes x.T and pre-transposed weight slices so all DMAs
are contiguous.  Matmuls run in float32r (full-rate fp32, ~2e-4 rel err).
Softmax is unnormalized with a fused ones-column in the v projection (zero
weights + bias 1) so the ctx matmul also emits the denominator; DVE divides
afterwards.  All PSUM usage cycles through two 2-bank [128,1024] tags so
exp() runs on 1024-wide tiles (halves ACT op count) and phases can overlap.
"""

import sys

for _p in ("/opt/trn_rl_repo",):
    if _p not in sys.path:
        sys.path.insert(0, _p)

import ml_dtypes
import numpy as np

import concourse.bass as bass
import concourse.mybir as mybir
import concourse.tile as tile
from concourse import bacc, bass_utils

F32 = mybir.dt.float32
F32R = mybir.dt.float32r
BF16 = mybir.dt.bfloat16
AF = mybir.ActivationFunctionType
ALU = mybir.AluOpType

N_CORES = 8
B, S, D, H = 2, 2048, 1024, 16
HG = 4              # heads per core
DK = 64             # head dim
F = HG * DK         # 256 features per core
FA = HG * (DK + 1)  # 260: v features + per-head denominator ones-column
DT = D // 128       # 8 d-tiles
FT = F // 128       # 2 f-tiles
ST = S // 128       # 16 s-tiles / k-tiles


def _build():
    nc = bacc.Bacc("TRN2", target_bir_lowering=False, debug=False,
                   num_devices=N_CORES)

    def din(name, shape, dt=F32):
        return nc.dram_tensor(name, shape, dt, kind="ExternalInput").ap()

    xqT = din("xqT", (D, S), BF16)
    xkT = din("xkT", (D, S), BF16)
    xvT = din("xvT", (D, S), BF16)
    wqT = din("wqT", (D, F), BF16)
    wkT = din("wkT", (D, F), BF16)
    wvT = din("wvT", (D, FA), BF16)   # interleaved, zero ones-columns
    woT = din("woT", (F, D), BF16)
    bq8 = din("bq8", (F, 1))
    bk = din("bk", (F, 1))
    bv260 = din("bv260", (128, FA))  # bv broadcast, 1.0 at ones-columns
    outT = nc.dram_tensor("outT", (D, S), BF16, kind="ExternalOutput").ap()

    with tile.TileContext(nc) as tc:
        with (
            tc.tile_pool(name="const", bufs=1) as cp,
            tc.tile_pool(name="data", bufs=1) as dp,
            tc.tile_pool(name="io", bufs=1) as iop,
            tc.tile_pool(name="dr", bufs=1, space="DRAM") as drp,
            tc.tile_pool(name="pp", bufs=2, space="PSUM") as pp,
        ):
            def psum_sc():
                # scores / general tag: 3 bufs x 2 banks = 6 banks
                return pp.tile([128, 1024], F32, name="sc", bufs=3)

            def psum_pc():
                # ctx-accumulator tag: 1 buf x 2 banks
                return pp.tile([128, 1024], F32, name="pc", bufs=1)

            # ---- PE warmup: ~6us of dep-free matmuls so HAM un-throttles
            # the PE clock (K=4/8 -> 8/8) before the first real matmul ----
            wmup = cp.tile([128, 512], BF16, name="wmup")
            nc.gpsimd.memset(wmup[:], 0.0)

            pw = psum_sc()
            for i in range(44):
                nc.tensor.matmul(pw[:, 0:512], wmup[:, 0:128], wmup[:],
                                 start=(i == 0), stop=True,
                                 skip_group_check=True)

            # ---- constants / weights (DMAs issued just before use) -------
            wq_t = [cp.tile([128, F], BF16, name=f"wq{d}") for d in range(DT)]
            wk_t = [cp.tile([128, F], BF16, name=f"wk{d}") for d in range(DT)]
            wv_t = [cp.tile([128, FA], BF16, name=f"wv{d}") for d in range(DT)]
            wo_t = [cp.tile([128, D], BF16, name=f"wo{t}") for t in range(FT)]
            bq8_t = [cp.tile([128, 1], F32, name=f"bq8{t}") for t in range(FT)]
            bk_t = [cp.tile([128, 1], F32, name=f"bk{t}") for t in range(FT)]
            bv_t = cp.tile([128, FA], F32, name="bv")
            for t in range(FT):
                nc.sync.dma_start(bq8_t[t][:], bq8[t * 128:(t + 1) * 128, :])
            for d in range(DT):
                nc.sync.dma_start(wq_t[d][:], wqT[d * 128:(d + 1) * 128, :])

            # tri01[p, y] = 1 if y >= p else 0  (keep k<=q on diagonal blocks)
            tri = cp.tile([128, 128], BF16, name="tri")
            nc.gpsimd.memset(tri[:], 1.0)
            nc.gpsimd.affine_select(
                out=tri[:], in_=tri[:], compare_op=ALU.is_ge,
                fill=0.0, base=0, pattern=[[1, 128]], channel_multiplier=-1)

            # ---- persistent per-core tensors -----------------------------
            qpT = [dp.tile([128, S], BF16, name=f"qpT{t}") for t in range(FT)]
            kpT = [dp.tile([128, S], BF16, name=f"kpT{t}") for t in range(FT)]
            vp = [dp.tile([128, FA], BF16, name=f"vp{st}") for st in range(ST)]
            ctxn = [dp.tile([128, S], BF16, name=f"ctxn{t}") for t in range(FT)]

            # ---- phase A: q/k projections  [f, s] = wT.T @ xT ------------
            for (xT, w_t, qk, is_q, bias_t) in (
                    (xqT, wq_t, qpT, True, bq8_t),
                    (xkT, wk_t, kpT, False, bk_t)):
                if not is_q:   # stream k weights while q projection runs
                    for t in range(FT):
                        nc.sync.dma_start(bk_t[t][:],
                                          bk[t * 128:(t + 1) * 128, :])
                    for d in range(DT):
                        nc.sync.dma_start(wk_t[d][:],
                                          wkT[d * 128:(d + 1) * 128, :])
                p4 = [psum_sc(), psum_sc(), psum_sc(), psum_pc()]
                streams = {(0, 0): [(p4[0][:, 0:512], 0), (p4[0][:, 512:1024], 512)],
                           (0, 1): [(p4[1][:, 0:512], 0), (p4[1][:, 512:1024], 512)],
                           (1, 0): [(p4[2][:, 0:512], 0), (p4[2][:, 512:1024], 512)],
                           (1, 1): [(p4[3][:, 0:512], 0), (p4[3][:, 512:1024], 512)]}
                for d in range(DT):
                    xd = iop.tile([128, S], BF16, name="xq", bufs=4)
                    nc.sync.dma_start(
                        xd[:], xT[d * 128:(d + 1) * 128, :])
                    for t in range(FT):
                        lhsT = w_t[d][:, t * 128:(t + 1) * 128]
                        for sp in range(2):          # 1024-wide s chunks
                            for sh in range(2):      # 512-wide halves
                                s = sp * 2 + sh
                                nc.tensor.matmul(
                                    streams[(t, sp)][sh][0],
                                    lhsT,
                                    xd[:, s * 512:(s + 1) * 512],
                                    start=(d == 0), stop=(d == DT - 1))
                for t in range(FT):
                    for sp in range(2):
                        if (t, sp) == (1, 1):
                            parts = streams[(t, sp)]
                        else:
                            parts = [(p4[{(0, 0): 0, (0, 1): 1, (1, 0): 2}[(t, sp)]][:], 0)]
                        for (pslice, co) in parts:
                            wdt = pslice.shape[-1]
                            dst = qk[t][:, sp * 1024 + co:sp * 1024 + co + wdt]
                            if is_q:
                                nc.vector.tensor_scalar(
                                    dst, pslice, 0.125, bias_t[t][:],
                                    op0=ALU.mult, op1=ALU.add)
                            else:
                                nc.vector.tensor_scalar_add(
                                    dst, pslice, bias_t[t][:])

            # ---- phase A: v projection  [s, f] natural + ones column -----
            nc.sync.dma_start(bv_t[:], bv260[:])
            for d in range(DT):
                nc.sync.dma_start(wv_t[d][:], wvT[d * 128:(d + 1) * 128, :])

            def v_half(half):
                pv4 = [psum_sc(), psum_sc(), psum_sc(), psum_pc()]
                pv = {s8: pv4[s8 // 2][:, (s8 % 2) * 512:(s8 % 2) * 512 + FA]
                      for s8 in range(8)}
                for d in range(DT):
                    xd = iop.tile([128, S // 2], BF16, name="xv", bufs=4)
                    nc.sync.dma_start(
                        xd[:], xvT[d * 128:(d + 1) * 128,
                                   half * 1024:(half + 1) * 1024])
                    for s8 in range(8):
                        nc.tensor.matmul(
                            pv[s8],
                            xd[:, s8 * 128:(s8 + 1) * 128],
                            wv_t[d][:],
                            start=(d == 0), stop=(d == DT - 1))
                for s8 in range(8):
                    st = half * 8 + s8
                    nc.vector.tensor_add(vp[st][:], pv[s8], bv_t[:])

            # ---- phase B: attention, 1024-wide q chunks ------------------
            for t in range(FT):
                nc.sync.dma_start(wo_t[t][:], woT[t * 128:(t + 1) * 128, :])
            den_all = [dp.tile([4, 1024], F32, name=f"den_all{i}")
                       for i in range(2)]
            rec_all = [dp.tile([4, 1024], F32, name=f"rec_all{i}")
                       for i in range(2)]

            def normalize(hp):
                # one head pair: den row j holds [head 2hp | head 2hp+1]
                nc.vector.reciprocal(rec_all[hp][:], den_all[hp][:])
                dstage = drp.tile([4, 1024], F32, name="dstage", bufs=2)
                nc.sync.dma_start(dstage[:], rec_all[hp][:])
                for j in range(4):
                    q0 = j * 512
                    bc = iop.tile([128, 1024], F32, name="bc", bufs=4)
                    nc.sync.dma_start(
                        bc[:], dstage[j:j + 1, :].partition_broadcast(128))
                    for half, off in ((0, 0), (1, 64)):
                        dst = ctxn[hp][off:off + 64, q0:q0 + 512]
                        nc.vector.tensor_mul(
                            dst, dst, bc[off:off + 64,
                                         half * 512:half * 512 + 512])

            # head PAIRS share one 2-bank scores tile: head 2t in bank 0
            # (cols 0:512), head 2t+1 in bank 1 (cols 512:1024).  The two
            # K=64 scores matmuls run CONCURRENTLY in disjoint PE row groups
            # (base partitions 0 / 64), and one exp() covers both heads.
            def attn_j(hp, j):
                    t = hp
                    h0, h1 = 2 * hp, 2 * hp + 1
                    q0 = j * 512
                    pc = psum_pc()              # bank0: head h0, bank1: h1
                    b_started = [False, False]

                    def ctx_mm(kt, ex, c0, last):
                        w = 512 - c0
                        for half, h in ((0, h0), (1, h1)):
                            nc.tensor.matmul(
                                pc[0:65, half * 512 + c0:half * 512 + 512],
                                vp[kt][:, 65 * h:65 * h + 65],
                                ex[:, half * 512:half * 512 + w],
                                start=not b_started[half], stop=last)
                            b_started[half] = True

                    kts = list(range(4 * j + 3, -1, -1))
                    pending = []
                    for kt in kts:
                        c0 = max(0, 128 * kt - q0)
                        w = 512 - c0
                        psc = psum_sc()
                        for half, off in ((0, 0), (1, 64)):
                            nc.tensor.matmul(
                                psc[:, half * 512:half * 512 + w],
                                kpT[t][off:off + 64,
                                       kt * 128:(kt + 1) * 128],
                                qpT[t][off:off + 64, q0 + c0:q0 + 512],
                                start=True, stop=True)
                        ex = iop.tile([128, 1024], BF16, name="ex", bufs=8)
                        if w == 512:
                            nc.scalar.activation(ex[:], psc[:], AF.Exp)
                        else:
                            v2 = psc[:].rearrange("p (b c) -> p b c",
                                                  c=512)[:, :, 0:w]
                            e2 = ex[:].rearrange("p (b c) -> p b c",
                                                 c=512)[:, :, 0:w]
                            nc.scalar.activation(e2, v2, AF.Exp)
                        if 128 * kt >= q0:   # diagonal: triangular mask
                            nc.vector.tensor_mul(ex[:, 0:128], ex[:, 0:128],
                                                 tri[:])
                            nc.vector.tensor_mul(ex[:, 512:640],
                                                 ex[:, 512:640], tri[:])
                        pending.append((kt, ex, c0))
                        if len(pending) > 2:
                            ctx_mm(*pending.pop(0), last=False)
                    while pending:
                        ctx_mm(*pending.pop(0), last=(len(pending) == 0))

                    # stash unnormalized ctx + denominator; normalize later
                    nc.vector.tensor_copy(
                        ctxn[t][0:64, q0:q0 + 512], pc[0:64, 0:512])
                    nc.vector.tensor_copy(
                        ctxn[t][64:128, q0:q0 + 512], pc[0:64, 512:1024])
                    dst = iop.tile([1, 1024], F32, name="denst", bufs=2)
                    nc.vector.tensor_copy(dst[:], pc[64:65, :])
                    # DVE/gpsimd can only address partition 0; DMA scatters
                    # the row into den_all's partition j for batching.
                    nc.sync.dma_start(den_all[hp][j:j + 1, :], dst[:])

            # interleave: v-half0 -> attention j0,j1 of pair 0 (needs only
            # vp[0..7]) -> v-half1 -> rest.  ACT starts ~12us earlier.
            v_half(0)
            attn_j(0, 0)
            attn_j(0, 1)
            v_half(1)
            attn_j(0, 2)
            attn_j(0, 3)
            normalize(0)
            for j in range(4):
                attn_j(1, j)
            normalize(1)

            # ---- phase C: output projection ------------------------------
            for e in range(DT):
                for sp in range(2):
                    po = psum_sc()
                    for t in range(FT):
                        for sh in range(2):
                            s = sp * 2 + sh
                            nc.tensor.matmul(
                                po[:, sh * 512:(sh + 1) * 512],
                                wo_t[t][:, e * 128:(e + 1) * 128],
                                ctxn[t][:, s * 512:(s + 1) * 512],
                                start=(t == 0), stop=(t == FT - 1))
                    ob = iop.tile([128, 1024], BF16, name="ob", bufs=3)
                    nc.scalar.copy(ob[:], po[:])
                    nc.sync.dma_start(
                        outT[e * 128:(e + 1) * 128,
                             sp * 1024:(sp + 1) * 1024],
                        ob[:])

    nc.compile()
    return nc


_NC_CACHE = {}


def _get_nc():
    if "nc" not in _NC_CACHE:
        _NC_CACHE["nc"] = _build()
    return _NC_CACHE["nc"]


def _in_maps(q, k, v, wq, bq, wk, bk, wv, bv, wo):
    maps = []
    xT = {}
    for b in range(B):
        xT[b] = tuple(np.ascontiguousarray(x[b].T).astype(ml_dtypes.bfloat16)
                      for x in (q, k, v))
    per_g = {}
    for g in range(HG):
        sl = slice(g * F, (g + 1) * F)
        # interleave v weights/bias with the denominator ones-column per head
        wv_aug = np.zeros((D, FA), np.float32)
        bv_aug = np.zeros((FA,), np.float32)
        wv_sl = wv[sl, :]
        bv_sl = bv[sl]
        for h in range(HG):
            wv_aug[:, h * 65:h * 65 + 64] = wv_sl[h * 64:(h + 1) * 64, :].T
            bv_aug[h * 65:h * 65 + 64] = bv_sl[h * 64:(h + 1) * 64]
            bv_aug[h * 65 + 64] = 1.0
        per_g[g] = dict(
            wqT=np.ascontiguousarray(wq[sl, :].T).astype(ml_dtypes.bfloat16),
            wkT=np.ascontiguousarray(wk[sl, :].T).astype(ml_dtypes.bfloat16),
            wvT=wv_aug.astype(ml_dtypes.bfloat16),
            woT=np.ascontiguousarray(wo[:, sl].T).astype(ml_dtypes.bfloat16),
            bq8=np.ascontiguousarray((bq[sl] / 8.0).reshape(F, 1)),
            bk=np.ascontiguousarray(bk[sl].reshape(F, 1)),
            bv260=np.ascontiguousarray(np.broadcast_to(bv_aug, (128, FA))),
        )
    for c in range(N_CORES):
        b, g = c // HG, c % HG
        m = dict(xqT=xT[b][0], xkT=xT[b][1], xvT=xT[b][2])
        m.update(per_g[g])
        maps.append(m)
    return maps


def run(inputs, trace=False, tmpdir=None):
    nc = _get_nc()
    q = np.asarray(inputs["q"], np.float32)
    k = np.asarray(inputs["k"], np.float32)
    v = np.asarray(inputs["v"], np.float32)
    maps = _in_maps(q, k, v,
                    np.asarray(inputs["wq"], np.float32),
                    np.asarray(inputs["bq"], np.float32),
                    np.asarray(inputs["wk"], np.float32),
                    np.asarray(inputs["bk"], np.float32),
                    np.asarray(inputs["wv"], np.float32),
                    np.asarray(inputs["bv"], np.float32),
                    np.asarray(inputs["wo"], np.float32))
    kwargs = {}
    if trace:
        kwargs = dict(trace=True, tmpdir=tmpdir)
    res = bass_utils.run_bass_kernel_spmd(
        nc, maps, core_ids=list(range(N_CORES)), **kwargs)
    bo = np.asarray(inputs["bo"], np.float32)
    out = np.empty((B, S, D), np.float32)
    for b in range(B):
        acc = res.results[4 * b]["outT"].astype(np.float32)
        for g in range(1, HG):
            acc += res.results[4 * b + g]["outT"].astype(np.float32)
        out[b] = acc.T + bo
    return out, res


def kernel(**inputs):
    out, _ = run(inputs)
    return out

